# revision 31
# baseline (speedup 1.0000x reference)
"""Block-sparse local+strided attention (LocalStridedBlockSparseAttn) on 8 trn2 cores.

q,k,v [4096, 16, 64] f32, single prefill sequence. Per-head block mask (64x64
token blocks): j <= i and (i - j < 8 or (j + h + 1) % 8 == 0). Core c owns
heads {c, c+8}; both share the strided residue s = (7 - c) % 8, so one SPMD
program serves all 8 cores with per-core data only.

v2 design (bf16, PE-lean):
  - Host pre-transposes q,k to [128 = 2 heads x 64 d, T] bf16 and pre-packs V
    with ones columns; normalization (divide by row sums) happens on host.
  - QK produces P^T [k-tokens, q] tiles directly: stationary = kT slices
    (d=64 on partitions -> the two heads run CONCURRENTLY on disjoint
    PE row groups via tile_position auto-derivation), moving = qT (N=512).
  - Local band processed as 32 k-pair windows: pair m (128 k-tokens) vs
    q cols [128m, 128m+512) in one N<=512 matmul + a [64,64] tail
    (j=2m+1 vs i=2m+8). Diagonal causal masking post-exp via one
    strided-AP multiply per chunk-head on gpsimd.
  - Strided part per 512-q chunk c: packed k-block pairs (N=512), the
    boundary block's partial validity handled by a -1e9 bias row input
    (per-core data) accumulated via a [1,*] matmul pre-exp.
  - exp split: ACT (exact, table exp) for strided/singles/edge mains;
    DVE computes exp via a Schraudolph bf16-bitcast (one tensor_scalar:
    int16(A*s + B) reinterpreted as bf16) for most local mains + tails.
  - PV in O^T layout: stationary Vaug = [V | 1] (65 cols), moving = P^T
    pieces, accumulated per chunk into PSUM [65, 512]; row 64 = softmax
    denominators. Copy to SBUF, DMA out; host divides and transposes.
"""

import numpy as np

N_HEADS = 16
HEAD = 64
SEQ = 4096
BS = 64
NB = 64
CHUNK = 512
NCH = SEQ // CHUNK   # 8
NCORES = 8
NPAIR = 32           # local k-pairs (128 tokens each)
SM_SCALE = 0.125

# Schraudolph exp in bf16 bit space: bf16bits(exp(s/8)) ~ int(A*s + B)
SCH_A = 128.0 / float(np.log(2.0)) * SM_SCALE      # 23.0831...
SCH_B = 16256.0 - 7.5 + 0.5                        # bias - RMS-centering + trunc comp

_cache = {}

import os
EXP_SPLIT_BANKS = os.environ.get("KBS_EXP_SPLIT", "0") == "1"   # per-bank exp APs
USE_SCHRAUDOLPH = os.environ.get("KBS_SCH", "1") == "1"         # DVE int16 exp
MASK_ON_GPSIMD = os.environ.get("KBS_GPSIMD", "1") == "1"       # gpsimd mask mult
NCH_RUN = int(os.environ.get("KBS_NCH", "8"))                   # chunks to emit
NO_PV = os.environ.get("KBS_NO_PV", "0") == "1"
NO_MASK = os.environ.get("KBS_NO_MASK", "0") == "1"
NO_TAILS = os.environ.get("KBS_NO_TAILS", "0") == "1"
NO_STR = os.environ.get("KBS_NO_STR", "0") == "1"
TAILS_H = os.environ.get("KBS_TAILS_H", "01")    # which heads get tails


def _legalize_waits(nc, max_waits=1):
    """This walrus build rejects instructions carrying more than one sync-wait
    condition; hoist extras into same-engine NoOps placed before the instr."""
    import concourse.mybir as mybir

    nid = 0
    for bb in nc.main_func.blocks:
        new = []
        for ins in bb.instructions:
            si = ins.sync_info
            if si is not None and si.on_wait and len(si.on_wait) > max_waits:
                waits = list(si.on_wait)
                while len(waits) > max_waits:
                    chunk, waits = waits[:max_waits], waits[max_waits:]
                    nid += 1
                    nop = mybir.InstNoOp(name=f"{ins.name}-wsplit{nid}")
                    nop.engine = ins.engine
                    nop.sync_info = mybir.SyncInfo(on_wait=chunk, on_update=[])
                    new.append(nop)
                ins.sync_info = mybir.SyncInfo(on_wait=waits,
                                               on_update=list(si.on_update))
            new.append(ins)
        bb.instructions[:] = new
    return nc


def _mask_np():
    """[128, 512] = 4 copies of the 128x128 diag-pair mask: rows 0-63 (even
    block j=2m) vs q blocks (2m, 2m+1): [tril-allow | ones]; rows 64-127
    (j=2m+1): [zeros | tril-allow]."""
    n = np.arange(64)
    tri = (n[None, :] >= n[:, None]).astype(np.float32)  # q >= k within block
    m = np.zeros((128, 128), np.float32)
    m[0:64, 0:64] = tri
    m[0:64, 64:128] = 1.0
    m[64:128, 64:128] = tri
    return np.tile(m, (1, 4))


def _build_program():
    from contextlib import ExitStack

    import concourse.bass as bass
    import concourse.mybir as mybir
    from concourse import tile

    f32 = mybir.dt.float32
    bf16 = mybir.dt.bfloat16
    i16 = mybir.dt.int16
    Exp = mybir.ActivationFunctionType.Exp
    MUL = mybir.AluOpType.mult
    ADD = mybir.AluOpType.add

    nc = bass.Bass()
    qh = nc.dram_tensor("qh", [8 * 128, 896], bf16, kind="ExternalInput")
    kh = nc.dram_tensor("kh", [8 * 128, 512], bf16, kind="ExternalInput")
    vbh = nc.dram_tensor("vbh", [8 * 128, 1040], bf16, kind="ExternalInput")
    ksh = nc.dram_tensor("ksh", [128, 448], bf16, kind="ExternalInput")
    vsh = nc.dram_tensor("vsh", [128, 1040], bf16, kind="ExternalInput")
    biasrow = nc.dram_tensor("biasrow", [1, 512], bf16, kind="ExternalInput")
    outh = nc.dram_tensor("outh", [128, SEQ], f32, kind="ExternalOutput")
    sumh = nc.dram_tensor("sumh", [2, SEQ], f32, kind="ExternalOutput")

    import ml_dtypes
    mask_d = nc.inline_tensor(_mask_np().astype(ml_dtypes.bfloat16), "mask_c")
    ones_d = nc.inline_tensor(np.ones((1, 128), ml_dtypes.bfloat16), "ones_c")

    with tile.TileContext(nc) as tc, ExitStack() as ctx:
        const = ctx.enter_context(tc.tile_pool(name="const", bufs=1))
        mask = const.tile([128, 512], bf16, tag="mask")
        ones = const.tile([1, 128], bf16, tag="ones")
        bias = const.tile([1, 512], bf16, tag="bias")
        kst = const.tile([128, 448], bf16, tag="kst")
        vst = const.tile([128, 1040], bf16, tag="vst")
        qpool = ctx.enter_context(tc.tile_pool(name="qts", bufs=8))
        kpool = ctx.enter_context(tc.tile_pool(name="kts", bufs=8))
        vpool = ctx.enter_context(tc.tile_pool(name="vbs", bufs=8))
        qts, kts, vbs = [], [], []
        for t in range(NCH):
            qts.append(qpool.tile([128, 896], bf16, tag="qts",
                                  name=f"qt{t}"))
            kts.append(kpool.tile([128, 512], bf16, tag="kts",
                                  name=f"kt{t}"))
            vbs.append(vpool.tile([128, 1040], bf16, tag="vbs",
                                  name=f"vt{t}"))
        # chunk-0 q/k first so the PE can start ASAP; consts mid-stream
        nc.sync.dma_start(qts[0][:], qh[0:128, :])
        nc.sync.dma_start(kts[0][:], kh[0:128, :])
        nc.sync.dma_start(mask[:], mask_d[:])
        nc.sync.dma_start(vbs[0][:], vbh[0:128, :])
        nc.sync.dma_start(qts[1][:], qh[128:256, :])
        nc.sync.dma_start(kts[1][:], kh[128:256, :])
        nc.sync.dma_start(ones[:], ones_d[:])
        nc.sync.dma_start(bias[:], biasrow[:])
        nc.sync.dma_start(kst[:], ksh[:])
        nc.sync.dma_start(vst[:], vsh[:])
        for t in range(2, NCH):
            nc.sync.dma_start(qts[t][:], qh[128 * t:128 * (t + 1), :])
            nc.sync.dma_start(kts[t][:], kh[128 * t:128 * (t + 1), :])
            nc.sync.dma_start(vbs[t - 1][:], vbh[128 * (t - 1):128 * t, :])
        nc.sync.dma_start(vbs[7][:], vbh[128 * 7:128 * 8, :])

        # SBUF P-hat pools
        pmpool = ctx.enter_context(tc.tile_pool(name="pmain", bufs=3))
        pspool_sb = ctx.enter_context(tc.tile_pool(name="pstr", bufs=2))
        pgpool_sb = ctx.enter_context(tc.tile_pool(name="psng", bufs=2))
        ptpool_sb = ctx.enter_context(tc.tile_pool(name="ptail", bufs=2))
        osbpool = ctx.enter_context(tc.tile_pool(name="osb", bufs=4))
        # PSUM pools
        psp = ctx.enter_context(tc.tile_pool(name="ps", bufs=2, space="PSUM"))
        otp = ctx.enter_context(tc.tile_pool(name="ot", bufs=4, space="PSUM"))
        # banks: ps 2x2 + ot 4x1 = 8 (tails share the ps slots)

        pmains = [None] * NCH   # [128, 4096] bf16: pair 4c+j at [1024j:+1024]
        ptails = [None] * NCH   # [128, 512] bf16 (rows 64-127 used)
        pstrs = [None] * NCH    # [128, 3072] bf16: stack p at [1024p:+1024]
        psngs = [None] * NCH    # [128, 512] bf16

        def emit_qk_mains(c, jlist):
            pm = pmains[c]
            for j in jlist:
                m = 4 * c + j
                W = min(512, SEQ - 128 * m)
                ps = psp.tile([128, 1024], f32, tag="ps", name=f"ps_m{m}")
                for h in (0, 1):
                    hp = slice(64 * h, 64 * h + 64)
                    nc.tensor.matmul(
                        ps[:, 512 * h:512 * h + W],
                        kts[c][hp, 128 * j:128 * j + 128],
                        qts[c][hp, 128 * j:128 * j + W],
                        start=True, stop=True, skip_group_check=True)
                if j < 3 and m <= 28 and USE_SCHRAUDOLPH:
                    if EXP_SPLIT_BANKS:
                        for h in (0, 1):
                            nc.vector.tensor_scalar(
                                pm[:, 1024 * j + 512 * h:1024 * j + 512 * h + 512].bitcast(i16),
                                ps[:, 512 * h:512 * h + 512],
                                SCH_A, SCH_B, MUL, ADD)
                    else:
                        dst = pm[:, 1024 * j:1024 * j + 1024]
                        nc.vector.tensor_scalar(dst.bitcast(i16), ps[:],
                                                SCH_A, SCH_B, MUL, ADD)
                elif W == 512:
                    if EXP_SPLIT_BANKS:
                        for h in (0, 1):
                            nc.scalar.activation(
                                pm[:, 1024 * j + 512 * h:1024 * j + 512 * h + 512],
                                ps[:, 512 * h:512 * h + 512], Exp, scale=SM_SCALE)
                    else:
                        dst = pm[:, 1024 * j:1024 * j + 1024]
                        nc.scalar.activation(dst, ps[:], Exp, scale=SM_SCALE)
                else:
                    # edge pairs: only [0:W] / [512:512+W] were written
                    for h in (0, 1):
                        nc.scalar.activation(
                            pm[:, 1024 * j + 512 * h:1024 * j + 512 * h + W],
                            ps[:, 512 * h:512 * h + W], Exp, scale=SM_SCALE)
                if not NO_MASK:
                    # zero the diag-masked regions (cols [0:128] of each
                    # head's half) in one 2D-strided multiply on DVE
                    src2 = pm[:, 1024 * j:1024 * j + 1024].rearrange(
                        "p (g w) -> p g w", w=512)[:, :, 0:128]
                    msk2 = mask[:, 0:256].rearrange("p (g w) -> p g w", w=128)
                    nc.vector.tensor_tensor(src2, src2, msk2, MUL)

        def emit_qk_rest(c):
            # strided pairs (blocks 2p, 2p+1 valid: 2p+1 <= c-1)
            npairs = c // 2
            for p in range(npairs):
                bnd = (c % 2 == 0) and (p == npairs - 1)   # block c-1 == 2p+1
                ps = psp.tile([128, 1024], f32, tag="ps", name=f"ps_s{c}_{p}")
                for h in (0, 1):
                    hp = slice(64 * h, 64 * h + 64)
                    nc.tensor.matmul(
                        ps[:, 512 * h:512 * h + 512],
                        kst[hp, 128 * p:128 * p + 128],
                        qts[c][hp, 0:512],
                        start=True, stop=not bnd, skip_group_check=True)
                    if bnd:
                        nc.tensor.matmul(
                            ps[64:128, 512 * h:512 * h + 512],
                            ones[0:1, 0:64], bias[:],
                            start=False, stop=True, skip_group_check=True)
                if EXP_SPLIT_BANKS:
                    for h in (0, 1):
                        nc.scalar.activation(
                            pstrs[c][:, 1024 * p + 512 * h:1024 * p + 512 * h + 512],
                            ps[:, 512 * h:512 * h + 512], Exp, scale=SM_SCALE)
                else:
                    nc.scalar.activation(pstrs[c][:, 1024 * p:1024 * p + 1024],
                                         ps[:], Exp, scale=SM_SCALE)
            if c % 2 == 1:
                # single strided block b = c-1: h0 in bank A rows 0-63,
                # h1 in bank B rows 64-127 (independent start=True per bank)
                b = c - 1
                pg = psp.tile([128, 1024], f32, tag="ps", name=f"ps_g{c}")
                for h in (0, 1):
                    hp = slice(64 * h, 64 * h + 64)
                    reg = pg[hp, 512 * h:512 * h + 512]
                    nc.tensor.matmul(reg, kst[hp, 64 * b:64 * b + 64],
                                     qts[c][hp, 0:512],
                                     start=True, stop=False,
                                     skip_group_check=True)
                    nc.tensor.matmul(reg, ones[0:1, 0:64], bias[:],
                                     start=False, stop=True,
                                     skip_group_check=True)
                    nc.scalar.activation(psngs[c][hp, :], reg, Exp,
                                         scale=SM_SCALE)

        def emit_qk_tails(c):
            # tails m = 4(c-1)+j: j(block 2m+1) vs i = 2m+8 (q-block in chunk c)
            # h0 tails rows 0-63 (tile (0,0)); h1 rows 64-127 (tile
            # (64,64)). start=True clears only the tile's own PSUM partition
            # range, so per-head start=True shares one bank safely.
            pt = psp.tile([128, 512], f32, tag="ps", name=f"pt{c}")
            from concourse.tile_rust import add_dep_helper
            hlist = [int(x) for x in TAILS_H]
            mm0h = {}
            for j in range(4):
                for h in hlist:
                    hp = slice(64 * h, 64 * h + 64)
                    col = 256 * h + 64 * j
                    mm = nc.tensor.matmul(
                        pt[hp, col:col + 64],
                        kts[c - 1][hp, 128 * j + 64:128 * j + 128],
                        qts[c][hp, 128 * j:128 * j + 64],
                        start=(j == 0), stop=(j == 3),
                        skip_group_check=True)
                    if j == 0:
                        mm0h[h] = mm
                    else:
                        # j=0's start=True clears the tile's psum range; the
                        # disjoint-region matmuls must not be scheduled first
                        add_dep_helper(mm.ins, mm0h[h].ins, sync=False,
                                       reason="tail has_written order")
            for h in hlist:
                hp = slice(64 * h, 64 * h + 64)
                dst = ptails[c][hp, 256 * h:256 * h + 256]
                src_ = pt[hp, 256 * h:256 * h + 256]
                if USE_SCHRAUDOLPH:
                    nc.vector.tensor_scalar(dst.bitcast(i16), src_,
                                            SCH_A, SCH_B, MUL, ADD)
                else:
                    nc.scalar.activation(dst, src_, Exp, scale=SM_SCALE)

        def emit_mask(c):
            # zero the masked diag regions of the 4 mains, one op per head
            pm = pmains[c]
            src = pm[:].rearrange("p (j w) -> p j w", w=1024)
            msk = mask[:].rearrange("p (j w) -> p j w", w=128)
            eng = nc.gpsimd if MASK_ON_GPSIMD else nc.vector
            for h in (0, 1):
                ap = src[:, :, 512 * h:512 * h + 128]
                eng.tensor_tensor(ap, ap, msk[:, :, :], MUL)

        def emit_pv(c, heads=(0, 1)):
            for h in heads:
                pieces = []   # (lhsT, rhs, out_col, W) - 512-wide ones first
                pm = pmains[c]
                # pair 4c covers the full chunk (chain-A opener)
                pieces.append((vbs[c][:, 65 * h:65 * h + 65],
                               pm[:, 512 * h:512 * h + 512], 0, 512))
                for p in range(c // 2 if not NO_STR else 0):
                    pieces.append((vst[:, 130 * p + 65 * h:130 * p + 65 * h + 65],
                                   pstrs[c][:, 1024 * p + 512 * h:1024 * p + 512 * h + 512],
                                   0, 512))
                if c % 2 == 1 and not NO_STR:
                    b = c - 1
                    if h == 0:
                        pieces.append((vst[0:64, 130 * (b // 2):130 * (b // 2) + 65],
                                       psngs[c][0:64, :], 0, 512))
                    else:
                        pieces.append((vst[64:128, 130 * (4 + b // 2) + 65:130 * (4 + b // 2) + 130],
                                       psngs[c][64:128, :], 0, 512))
                for j in range(1, 4):
                    m = 4 * c + j
                    if m > 31:
                        continue
                    W = 512 - 128 * j
                    pieces.append((vbs[c][:, 130 * j + 65 * h:130 * j + 65 * h + 65],
                                   pm[:, 1024 * j + 512 * h:1024 * j + 512 * h + W],
                                   128 * j, W))
                if c >= 1:
                    pmp = pmains[c - 1]
                    for j in range(1, 4):
                        W = 128 * j
                        off = 1024 * j + 512 * h + (512 - W)
                        pieces.append((vbs[c - 1][:, 130 * j + 65 * h:130 * j + 65 * h + 65],
                                       pmp[:, off:off + W], 0, W))
                    tails_on = (not NO_TAILS) and str(h) in TAILS_H
                    for j in range(4 if tails_on else 0):  # tails
                        if h == 0:
                            pieces.append((vbs[c - 1][0:64, 520 + 130 * j:520 + 130 * j + 65],
                                           ptails[c][0:64, 64 * j:64 * j + 64],
                                           128 * j, 64))
                        else:
                            pieces.append((vbs[c - 1][64:128, 130 * j + 65:130 * j + 130],
                                           ptails[c][64:128, 256 + 64 * j:256 + 64 * j + 64],
                                           128 * j, 64))
                # alternate pieces across two accumulator banks so consecutive
                # accumulating matmuls never chain through the same PSUM bank
                two = pieces[1][3] == 512 and len(pieces) > 4
                ota = otp.tile([65, 512], f32, tag="ot", name=f"ot{c}_{h}a")
                otb = otp.tile([65, 512], f32, tag="ot", name=f"ot{c}_{h}b") \
                    if two else None
                chains = ([], [])
                for pi, pc_ in enumerate(pieces):
                    chains[pi % 2 if two else 0].append(pc_)
                # emit interleaved A0,B0,A1,B1,... so consecutive PE matmuls
                # target different banks and their drains overlap
                for pi, (vl, rh, col, W) in enumerate(pieces):
                    ci = pi % 2 if two else 0
                    tgt = (ota, otb)[ci]
                    chain = chains[ci]
                    idx = pi // 2 if two else pi
                    nc.tensor.matmul(tgt[:, col:col + W], vl, rh,
                                     start=(idx == 0),
                                     stop=(idx == len(chain) - 1),
                                     skip_group_check=True)
                osb = osbpool.tile([65, 512], f32, tag="osb",
                                   name=f"osb{c}_{h}")
                if two:
                    # only one PSUM operand allowed per instruction
                    nc.scalar.copy(osb[:], ota[:])
                    nc.vector.tensor_tensor(osb[:], osb[:], otb[:], ADD)
                else:
                    nc.any.tensor_copy(osb[:], ota[:])
                nc.sync.dma_start(outh[64 * h:64 * h + 64,
                                       512 * c:512 * c + 512], osb[0:64, :])
                nc.sync.dma_start(sumh[h:h + 1, 512 * c:512 * c + 512],
                                  osb[64:65, :])

        for c in range(NCH_RUN):
            pmains[c] = pmpool.tile([128, 4096], bf16, tag="pmain",
                                    name=f"pm{c}")
            if c >= 1:
                pstrs[c] = pspool_sb.tile([128, 3072], bf16, tag="pstr",
                                          name=f"pstr{c}")
                ptails[c] = ptpool_sb.tile([128, 512], bf16, tag="ptail",
                                           name=f"ptl{c}")
            if c % 2 == 1:
                psngs[c] = pgpool_sb.tile([128, 512], bf16, tag="psng",
                                          name=f"psg{c}")
            emit_qk_mains(c, (0, 1))
            if c >= 1 and not NO_PV:
                emit_pv(c - 1, (0,))
            emit_qk_mains(c, (2, 3))
            if c >= 1 and not NO_PV:
                emit_pv(c - 1, (1,))
            if c >= 1 and not NO_TAILS:
                emit_qk_tails(c)
            if not NO_STR:
                emit_qk_rest(c)

        if not NO_PV:
            emit_pv(NCH_RUN - 1)
        _cache["dbg_tiles"] = {"pmains": pmains, "ptails": ptails,
                               "pstrs": pstrs, "psngs": psngs,
                               "qts": qts, "kts": kts, "vbs": vbs,
                               "kst": kst, "vst": vst}

    return nc


def _in_maps(q, k, v):
    import ml_dtypes
    bf = ml_dtypes.bfloat16

    maps = []
    for c in range(NCORES):
        h0, h1 = c, c + 8
        s = (7 - c) % 8

        qT = np.concatenate([q[:, h0, :].T, q[:, h1, :].T], 0)  # [128, 4096]
        kT = np.concatenate([k[:, h0, :].T, k[:, h1, :].T], 0)
        qhm = np.zeros((1024, 896), np.float32)
        khm = np.zeros((1024, 512), np.float32)
        for t in range(8):
            w = min(896, SEQ - 512 * t)
            qhm[128 * t:128 * t + 128, :w] = qT[:, 512 * t:512 * t + w]
            khm[128 * t:128 * t + 128, :] = kT[:, 512 * t:512 * t + 512]

        vbm = np.zeros((1024, 1040), np.float32)
        for t in range(8):
            for a in range(4):
                blk = v[128 * (4 * t + a):128 * (4 * t + a) + 128, :, :]
                vbm[128 * t:128 * t + 128, 130 * a:130 * a + 64] = blk[:, h0, :]
                vbm[128 * t:128 * t + 128, 130 * a + 64] = 1.0
                vbm[128 * t:128 * t + 128, 130 * a + 65:130 * a + 129] = blk[:, h1, :]
                vbm[128 * t:128 * t + 128, 130 * a + 129] = 1.0
                # odd-block (tokens 64-127 of the pair) V dup at
                # partitions 0-63 for both heads (tail PV at col-group 0)
                col = 520 + 130 * a
                vbm[128 * t:128 * t + 64, col:col + 64] = blk[64:128, h0, :]
                vbm[128 * t:128 * t + 64, col + 64] = 1.0
                vbm[128 * t:128 * t + 64, col + 65:col + 129] = blk[64:128, h1, :]
                vbm[128 * t:128 * t + 64, col + 129] = 1.0

        ksm = np.zeros((128, 448), np.float32)
        for b in range(7):
            j = s + 8 * b
            ksm[0:64, 64 * b:64 * b + 64] = k[64 * j:64 * j + 64, h0, :].T
            ksm[64:128, 64 * b:64 * b + 64] = k[64 * j:64 * j + 64, h1, :].T

        vsm = np.zeros((128, 1040), np.float32)
        for p in range(4):  # pair 3 = lone block 6 on partitions 0-63
            halves = ((0, s + 16 * p), (1, s + 16 * p + 8)) if p < 3 \
                else ((0, s + 48),)
            for half, j in halves:
                r = slice(64 * half, 64 * half + 64)
                vsm[r, 130 * p:130 * p + 64] = v[64 * j:64 * j + 64, h0, :]
                vsm[r, 130 * p + 64] = 1.0
                vsm[r, 130 * p + 65:130 * p + 129] = v[64 * j:64 * j + 64, h1, :]
                vsm[r, 130 * p + 129] = 1.0
        for d in range(4):  # dup of even blocks (singles) on partitions 64-127
            j = s + 16 * d
            col = 130 * (4 + d)
            vsm[64:128, col:col + 64] = v[64 * j:64 * j + 64, h0, :]
            vsm[64:128, col + 64] = 1.0
            vsm[64:128, col + 65:col + 129] = v[64 * j:64 * j + 64, h1, :]
            vsm[64:128, col + 129] = 1.0

        biasm = np.zeros((1, 512), np.float32)
        biasm[0, :64 * s] = -1e9

        maps.append({"qh": qhm.astype(bf), "kh": khm.astype(bf),
                     "vbh": vbm.astype(bf), "ksh": ksm.astype(bf),
                     "vsh": vsm.astype(bf), "biasrow": biasm.astype(bf)})
    return maps


def kernel(q, k, v, cu_seqlens_k=None, **_):
    from concourse.bass_utils import run_bass_kernel_spmd

    q = np.asarray(q, np.float32)
    k = np.asarray(k, np.float32)
    v = np.asarray(v, np.float32)
    if "nc" not in _cache:
        _cache["nc"] = _legalize_waits(_build_program())
    res = run_bass_kernel_spmd(_cache["nc"], _in_maps(q, k, v),
                               list(range(NCORES))).results
    out = np.empty((SEQ, N_HEADS, HEAD), np.float32)
    for c in range(NCORES):
        o = np.asarray(res[c]["outh"], np.float32)
        sm = np.asarray(res[c]["sumh"], np.float32)
        out[:, c, :] = (o[0:64, :] / sm[0:1, :]).T
        out[:, c + 8, :] = (o[64:128, :] / sm[1:2, :]).T
    return out


# revision 32
# speedup vs baseline: 1.0388x; 1.0388x over previous
"""Block-sparse local+strided attention (LocalStridedBlockSparseAttn) on 8 trn2 cores.

q,k,v [4096, 16, 64] f32, single prefill sequence. Per-head block mask (64x64
token blocks): j <= i and (i - j < 8 or (j + h + 1) % 8 == 0). Core c owns
heads {c, c+8}; both share the strided residue s = (7 - c) % 8, so one SPMD
program serves all 8 cores with per-core data only.

v2 design (bf16, PE-lean):
  - Host pre-transposes q,k to [128 = 2 heads x 64 d, T] bf16 and pre-packs V
    with ones columns; normalization (divide by row sums) happens on host.
  - QK produces P^T [k-tokens, q] tiles directly: stationary = kT slices
    (d=64 on partitions -> the two heads run CONCURRENTLY on disjoint
    PE row groups via tile_position auto-derivation), moving = qT (N=512).
  - Local band processed as 32 k-pair windows: pair m (128 k-tokens) vs
    q cols [128m, 128m+512) in one N<=512 matmul + a [64,64] tail
    (j=2m+1 vs i=2m+8). Diagonal causal masking post-exp via one
    strided-AP multiply per chunk-head on gpsimd.
  - Strided part per 512-q chunk c: packed k-block pairs (N=512), the
    boundary block's partial validity handled by a -1e9 bias row input
    (per-core data) accumulated via a [1,*] matmul pre-exp.
  - exp split: ACT (exact, table exp) for strided/singles/edge mains;
    DVE computes exp via a Schraudolph bf16-bitcast (one tensor_scalar:
    int16(A*s + B) reinterpreted as bf16) for most local mains + tails.
  - PV in O^T layout: stationary Vaug = [V | 1] (65 cols), moving = P^T
    pieces, accumulated per chunk into PSUM [65, 512]; row 64 = softmax
    denominators. Copy to SBUF, DMA out; host divides and transposes.
"""

import numpy as np

N_HEADS = 16
HEAD = 64
SEQ = 4096
BS = 64
NB = 64
CHUNK = 512
NCH = SEQ // CHUNK   # 8
NCORES = 8
NPAIR = 32           # local k-pairs (128 tokens each)
SM_SCALE = 0.125

# Schraudolph exp in bf16 bit space: bf16bits(exp(s/8)) ~ int(A*s + B)
SCH_A = 128.0 / float(np.log(2.0)) * SM_SCALE      # 23.0831...
SCH_B = 16256.0 - 7.5 + 0.5                        # bias - RMS-centering + trunc comp

_cache = {}

import os
EXP_SPLIT_BANKS = os.environ.get("KBS_EXP_SPLIT", "0") == "1"   # per-bank exp APs
USE_SCHRAUDOLPH = os.environ.get("KBS_SCH", "1") == "1"         # DVE int16 exp
MASK_ON_GPSIMD = os.environ.get("KBS_GPSIMD", "1") == "1"       # gpsimd mask mult
NCH_RUN = int(os.environ.get("KBS_NCH", "8"))                   # chunks to emit
NO_PV = os.environ.get("KBS_NO_PV", "0") == "1"
NO_MASK = os.environ.get("KBS_NO_MASK", "0") == "1"
NO_TAILS = os.environ.get("KBS_NO_TAILS", "0") == "1"
NO_STR = os.environ.get("KBS_NO_STR", "0") == "1"
TAILS_H = os.environ.get("KBS_TAILS_H", "01")    # which heads get tails


def _legalize_waits(nc, max_waits=1):
    """This walrus build rejects instructions carrying more than one sync-wait
    condition; hoist extras into same-engine NoOps placed before the instr."""
    import concourse.mybir as mybir

    nid = 0
    for bb in nc.main_func.blocks:
        new = []
        for ins in bb.instructions:
            si = ins.sync_info
            if si is not None and si.on_wait and len(si.on_wait) > max_waits:
                waits = list(si.on_wait)
                while len(waits) > max_waits:
                    chunk, waits = waits[:max_waits], waits[max_waits:]
                    nid += 1
                    nop = mybir.InstNoOp(name=f"{ins.name}-wsplit{nid}")
                    nop.engine = ins.engine
                    nop.sync_info = mybir.SyncInfo(on_wait=chunk, on_update=[])
                    new.append(nop)
                ins.sync_info = mybir.SyncInfo(on_wait=waits,
                                               on_update=list(si.on_update))
            new.append(ins)
        bb.instructions[:] = new
    return nc


def _mask_np():
    """[128, 512] = 4 copies of the 128x128 diag-pair mask: rows 0-63 (even
    block j=2m) vs q blocks (2m, 2m+1): [tril-allow | ones]; rows 64-127
    (j=2m+1): [zeros | tril-allow]."""
    n = np.arange(64)
    tri = (n[None, :] >= n[:, None]).astype(np.float32)  # q >= k within block
    m = np.zeros((128, 128), np.float32)
    m[0:64, 0:64] = tri
    m[0:64, 64:128] = 1.0
    m[64:128, 64:128] = tri
    return np.tile(m, (1, 4))


def _build_program():
    from contextlib import ExitStack

    import concourse.bass as bass
    import concourse.mybir as mybir
    from concourse import tile

    f32 = mybir.dt.float32
    bf16 = mybir.dt.bfloat16
    i16 = mybir.dt.int16
    Exp = mybir.ActivationFunctionType.Exp
    MUL = mybir.AluOpType.mult
    ADD = mybir.AluOpType.add

    nc = bass.Bass()
    qh = nc.dram_tensor("qh", [8 * 128, 896], bf16, kind="ExternalInput")
    kh = nc.dram_tensor("kh", [8 * 128, 512], bf16, kind="ExternalInput")
    vbh = nc.dram_tensor("vbh", [8 * 128, 1040], bf16, kind="ExternalInput")
    ksh = nc.dram_tensor("ksh", [128, 448], bf16, kind="ExternalInput")
    vsh = nc.dram_tensor("vsh", [128, 1040], bf16, kind="ExternalInput")
    biasrow = nc.dram_tensor("biasrow", [1, 512], bf16, kind="ExternalInput")
    outh = nc.dram_tensor("outh", [128, SEQ], f32, kind="ExternalOutput")
    sumh = nc.dram_tensor("sumh", [2, SEQ], f32, kind="ExternalOutput")

    import ml_dtypes
    mask_d = nc.inline_tensor(_mask_np().astype(ml_dtypes.bfloat16), "mask_c")
    ones_d = nc.inline_tensor(np.ones((1, 128), ml_dtypes.bfloat16), "ones_c")

    with tile.TileContext(nc) as tc, ExitStack() as ctx:
        const = ctx.enter_context(tc.tile_pool(name="const", bufs=1))
        mask = const.tile([128, 512], bf16, tag="mask")
        ones = const.tile([1, 128], bf16, tag="ones")
        bias = const.tile([1, 512], bf16, tag="bias")
        kst = const.tile([128, 448], bf16, tag="kst")
        vst = const.tile([128, 1040], bf16, tag="vst")
        qpool = ctx.enter_context(tc.tile_pool(name="qts", bufs=8))
        kpool = ctx.enter_context(tc.tile_pool(name="kts", bufs=8))
        vpool = ctx.enter_context(tc.tile_pool(name="vbs", bufs=8))
        qts, kts, vbs = [], [], []
        for t in range(NCH):
            qts.append(qpool.tile([128, 896], bf16, tag="qts",
                                  name=f"qt{t}"))
            kts.append(kpool.tile([128, 512], bf16, tag="kts",
                                  name=f"kt{t}"))
            vbs.append(vpool.tile([128, 1040], bf16, tag="vbs",
                                  name=f"vt{t}"))
        # chunk-0 q/k first so the PE can start ASAP; consts mid-stream
        nc.gpsimd.dma_start(qts[0][:], qh[0:128, :])
        nc.gpsimd.dma_start(kts[0][:], kh[0:128, :])
        nc.gpsimd.dma_start(mask[:], mask_d[:])
        nc.gpsimd.dma_start(vbs[0][:], vbh[0:128, :])
        nc.gpsimd.dma_start(qts[1][:], qh[128:256, :])
        nc.gpsimd.dma_start(kts[1][:], kh[128:256, :])
        nc.gpsimd.dma_start(ones[:], ones_d[:])
        nc.gpsimd.dma_start(bias[:], biasrow[:])
        nc.gpsimd.dma_start(kst[:], ksh[:])
        nc.gpsimd.dma_start(vst[:], vsh[:])
        for t in range(2, NCH):
            nc.gpsimd.dma_start(qts[t][:], qh[128 * t:128 * (t + 1), :])
            nc.gpsimd.dma_start(kts[t][:], kh[128 * t:128 * (t + 1), :])
            nc.gpsimd.dma_start(vbs[t - 1][:], vbh[128 * (t - 1):128 * t, :])
        nc.gpsimd.dma_start(vbs[7][:], vbh[128 * 7:128 * 8, :])

        # SBUF P-hat pools
        pmpool = ctx.enter_context(tc.tile_pool(name="pmain", bufs=3))
        pspool_sb = ctx.enter_context(tc.tile_pool(name="pstr", bufs=2))
        pgpool_sb = ctx.enter_context(tc.tile_pool(name="psng", bufs=2))
        ptpool_sb = ctx.enter_context(tc.tile_pool(name="ptail", bufs=2))
        osbpool = ctx.enter_context(tc.tile_pool(name="osb", bufs=4))
        # PSUM pools
        psp = ctx.enter_context(tc.tile_pool(name="ps", bufs=2, space="PSUM"))
        otp = ctx.enter_context(tc.tile_pool(name="ot", bufs=4, space="PSUM"))
        # banks: ps 2x2 + ot 4x1 = 8 (tails share the ps slots)

        pmains = [None] * NCH   # [128, 4096] bf16: pair 4c+j at [1024j:+1024]
        ptails = [None] * NCH   # [128, 512] bf16 (rows 64-127 used)
        pstrs = [None] * NCH    # [128, 3072] bf16: stack p at [1024p:+1024]
        psngs = [None] * NCH    # [128, 512] bf16

        def emit_qk_mains(c, jlist):
            pm = pmains[c]
            for j in jlist:
                m = 4 * c + j
                W = min(512, SEQ - 128 * m)
                ps = psp.tile([128, 1024], f32, tag="ps", name=f"ps_m{m}")
                for h in (0, 1):
                    hp = slice(64 * h, 64 * h + 64)
                    nc.tensor.matmul(
                        ps[:, 512 * h:512 * h + W],
                        kts[c][hp, 128 * j:128 * j + 128],
                        qts[c][hp, 128 * j:128 * j + W],
                        start=True, stop=True, skip_group_check=True)
                if j < 3 and m <= 28 and USE_SCHRAUDOLPH:
                    if EXP_SPLIT_BANKS:
                        for h in (0, 1):
                            nc.vector.tensor_scalar(
                                pm[:, 1024 * j + 512 * h:1024 * j + 512 * h + 512].bitcast(i16),
                                ps[:, 512 * h:512 * h + 512],
                                SCH_A, SCH_B, MUL, ADD)
                    else:
                        dst = pm[:, 1024 * j:1024 * j + 1024]
                        nc.vector.tensor_scalar(dst.bitcast(i16), ps[:],
                                                SCH_A, SCH_B, MUL, ADD)
                elif W == 512:
                    if EXP_SPLIT_BANKS:
                        for h in (0, 1):
                            nc.scalar.activation(
                                pm[:, 1024 * j + 512 * h:1024 * j + 512 * h + 512],
                                ps[:, 512 * h:512 * h + 512], Exp, scale=SM_SCALE)
                    else:
                        dst = pm[:, 1024 * j:1024 * j + 1024]
                        nc.scalar.activation(dst, ps[:], Exp, scale=SM_SCALE)
                else:
                    # edge pairs: only [0:W] / [512:512+W] were written
                    for h in (0, 1):
                        nc.scalar.activation(
                            pm[:, 1024 * j + 512 * h:1024 * j + 512 * h + W],
                            ps[:, 512 * h:512 * h + W], Exp, scale=SM_SCALE)
                if not NO_MASK:
                    # zero the diag-masked regions (cols [0:128] of each
                    # head's half) in one 2D-strided multiply on DVE
                    src2 = pm[:, 1024 * j:1024 * j + 1024].rearrange(
                        "p (g w) -> p g w", w=512)[:, :, 0:128]
                    msk2 = mask[:, 0:256].rearrange("p (g w) -> p g w", w=128)
                    nc.vector.tensor_tensor(src2, src2, msk2, MUL)

        def emit_qk_rest(c):
            # strided pairs (blocks 2p, 2p+1 valid: 2p+1 <= c-1)
            npairs = c // 2
            for p in range(npairs):
                bnd = (c % 2 == 0) and (p == npairs - 1)   # block c-1 == 2p+1
                ps = psp.tile([128, 1024], f32, tag="ps", name=f"ps_s{c}_{p}")
                for h in (0, 1):
                    hp = slice(64 * h, 64 * h + 64)
                    nc.tensor.matmul(
                        ps[:, 512 * h:512 * h + 512],
                        kst[hp, 128 * p:128 * p + 128],
                        qts[c][hp, 0:512],
                        start=True, stop=not bnd, skip_group_check=True)
                    if bnd:
                        nc.tensor.matmul(
                            ps[64:128, 512 * h:512 * h + 512],
                            ones[0:1, 0:64], bias[:],
                            start=False, stop=True, skip_group_check=True)
                if EXP_SPLIT_BANKS:
                    for h in (0, 1):
                        nc.scalar.activation(
                            pstrs[c][:, 1024 * p + 512 * h:1024 * p + 512 * h + 512],
                            ps[:, 512 * h:512 * h + 512], Exp, scale=SM_SCALE)
                else:
                    nc.scalar.activation(pstrs[c][:, 1024 * p:1024 * p + 1024],
                                         ps[:], Exp, scale=SM_SCALE)
            if c % 2 == 1:
                # single strided block b = c-1: h0 in bank A rows 0-63,
                # h1 in bank B rows 64-127 (independent start=True per bank)
                b = c - 1
                pg = psp.tile([128, 1024], f32, tag="ps", name=f"ps_g{c}")
                for h in (0, 1):
                    hp = slice(64 * h, 64 * h + 64)
                    reg = pg[hp, 512 * h:512 * h + 512]
                    nc.tensor.matmul(reg, kst[hp, 64 * b:64 * b + 64],
                                     qts[c][hp, 0:512],
                                     start=True, stop=False,
                                     skip_group_check=True)
                    nc.tensor.matmul(reg, ones[0:1, 0:64], bias[:],
                                     start=False, stop=True,
                                     skip_group_check=True)
                    nc.scalar.activation(psngs[c][hp, :], reg, Exp,
                                         scale=SM_SCALE)

        def emit_qk_tails(c):
            # tails m = 4(c-1)+j: j(block 2m+1) vs i = 2m+8 (q-block in chunk c)
            # h0 tails rows 0-63 (tile (0,0)); h1 rows 64-127 (tile
            # (64,64)). start=True clears only the tile's own PSUM partition
            # range, so per-head start=True shares one bank safely.
            pt = psp.tile([128, 512], f32, tag="ps", name=f"pt{c}")
            from concourse.tile_rust import add_dep_helper
            hlist = [int(x) for x in TAILS_H]
            mm0h = {}
            for j in range(4):
                for h in hlist:
                    hp = slice(64 * h, 64 * h + 64)
                    col = 256 * h + 64 * j
                    mm = nc.tensor.matmul(
                        pt[hp, col:col + 64],
                        kts[c - 1][hp, 128 * j + 64:128 * j + 128],
                        qts[c][hp, 128 * j:128 * j + 64],
                        start=(j == 0), stop=(j == 3),
                        skip_group_check=True)
                    if j == 0:
                        mm0h[h] = mm
                    else:
                        # j=0's start=True clears the tile's psum range; the
                        # disjoint-region matmuls must not be scheduled first
                        add_dep_helper(mm.ins, mm0h[h].ins, sync=False,
                                       reason="tail has_written order")
            for h in hlist:
                hp = slice(64 * h, 64 * h + 64)
                dst = ptails[c][hp, 256 * h:256 * h + 256]
                src_ = pt[hp, 256 * h:256 * h + 256]
                if USE_SCHRAUDOLPH:
                    nc.vector.tensor_scalar(dst.bitcast(i16), src_,
                                            SCH_A, SCH_B, MUL, ADD)
                else:
                    nc.scalar.activation(dst, src_, Exp, scale=SM_SCALE)

        def emit_mask(c):
            # zero the masked diag regions of the 4 mains, one op per head
            pm = pmains[c]
            src = pm[:].rearrange("p (j w) -> p j w", w=1024)
            msk = mask[:].rearrange("p (j w) -> p j w", w=128)
            eng = nc.gpsimd if MASK_ON_GPSIMD else nc.vector
            for h in (0, 1):
                ap = src[:, :, 512 * h:512 * h + 128]
                eng.tensor_tensor(ap, ap, msk[:, :, :], MUL)

        def emit_pv(c, heads=(0, 1)):
            for h in heads:
                pieces = []   # (lhsT, rhs, out_col, W) - 512-wide ones first
                pm = pmains[c]
                # pair 4c covers the full chunk (chain-A opener)
                pieces.append((vbs[c][:, 65 * h:65 * h + 65],
                               pm[:, 512 * h:512 * h + 512], 0, 512))
                for p in range(c // 2 if not NO_STR else 0):
                    pieces.append((vst[:, 130 * p + 65 * h:130 * p + 65 * h + 65],
                                   pstrs[c][:, 1024 * p + 512 * h:1024 * p + 512 * h + 512],
                                   0, 512))
                if c % 2 == 1 and not NO_STR:
                    b = c - 1
                    if h == 0:
                        pieces.append((vst[0:64, 130 * (b // 2):130 * (b // 2) + 65],
                                       psngs[c][0:64, :], 0, 512))
                    else:
                        pieces.append((vst[64:128, 130 * (4 + b // 2) + 65:130 * (4 + b // 2) + 130],
                                       psngs[c][64:128, :], 0, 512))
                for j in range(1, 4):
                    m = 4 * c + j
                    if m > 31:
                        continue
                    W = 512 - 128 * j
                    pieces.append((vbs[c][:, 130 * j + 65 * h:130 * j + 65 * h + 65],
                                   pm[:, 1024 * j + 512 * h:1024 * j + 512 * h + W],
                                   128 * j, W))
                if c >= 1:
                    pmp = pmains[c - 1]
                    for j in range(1, 4):
                        W = 128 * j
                        off = 1024 * j + 512 * h + (512 - W)
                        pieces.append((vbs[c - 1][:, 130 * j + 65 * h:130 * j + 65 * h + 65],
                                       pmp[:, off:off + W], 0, W))
                    tails_on = (not NO_TAILS) and str(h) in TAILS_H
                    for j in range(4 if tails_on else 0):  # tails
                        if h == 0:
                            pieces.append((vbs[c - 1][0:64, 520 + 130 * j:520 + 130 * j + 65],
                                           ptails[c][0:64, 64 * j:64 * j + 64],
                                           128 * j, 64))
                        else:
                            pieces.append((vbs[c - 1][64:128, 130 * j + 65:130 * j + 130],
                                           ptails[c][64:128, 256 + 64 * j:256 + 64 * j + 64],
                                           128 * j, 64))
                # alternate pieces across two accumulator banks so consecutive
                # accumulating matmuls never chain through the same PSUM bank
                two = False  # bank-alternation regressed; single chain
                ota = otp.tile([65, 512], f32, tag="ot", name=f"ot{c}_{h}a")
                otb = otp.tile([65, 512], f32, tag="ot", name=f"ot{c}_{h}b") \
                    if two else None
                chains = ([], [])
                for pi, pc_ in enumerate(pieces):
                    chains[pi % 2 if two else 0].append(pc_)
                # emit interleaved A0,B0,A1,B1,... so consecutive PE matmuls
                # target different banks and their drains overlap
                for pi, (vl, rh, col, W) in enumerate(pieces):
                    ci = pi % 2 if two else 0
                    tgt = (ota, otb)[ci]
                    chain = chains[ci]
                    idx = pi // 2 if two else pi
                    nc.tensor.matmul(tgt[:, col:col + W], vl, rh,
                                     start=(idx == 0),
                                     stop=(idx == len(chain) - 1),
                                     skip_group_check=True)
                osb = osbpool.tile([65, 512], f32, tag="osb",
                                   name=f"osb{c}_{h}")
                if two:
                    # only one PSUM operand allowed per instruction
                    nc.scalar.copy(osb[:], ota[:])
                    nc.vector.tensor_tensor(osb[:], osb[:], otb[:], ADD)
                else:
                    nc.any.tensor_copy(osb[:], ota[:])
                nc.sync.dma_start(outh[64 * h:64 * h + 64,
                                       512 * c:512 * c + 512], osb[0:64, :])
                nc.sync.dma_start(sumh[h:h + 1, 512 * c:512 * c + 512],
                                  osb[64:65, :])

        for c in range(NCH_RUN):
            pmains[c] = pmpool.tile([128, 4096], bf16, tag="pmain",
                                    name=f"pm{c}")
            if c >= 1:
                pstrs[c] = pspool_sb.tile([128, 3072], bf16, tag="pstr",
                                          name=f"pstr{c}")
                ptails[c] = ptpool_sb.tile([128, 512], bf16, tag="ptail",
                                           name=f"ptl{c}")
            if c % 2 == 1:
                psngs[c] = pgpool_sb.tile([128, 512], bf16, tag="psng",
                                          name=f"psg{c}")
            emit_qk_mains(c, (0, 1))
            if c >= 1 and not NO_PV:
                emit_pv(c - 1, (0,))
            emit_qk_mains(c, (2, 3))
            if c >= 1 and not NO_PV:
                emit_pv(c - 1, (1,))
            if c >= 1 and not NO_TAILS:
                emit_qk_tails(c)
            if not NO_STR:
                emit_qk_rest(c)

        if not NO_PV:
            emit_pv(NCH_RUN - 1)
        _cache["dbg_tiles"] = {"pmains": pmains, "ptails": ptails,
                               "pstrs": pstrs, "psngs": psngs,
                               "qts": qts, "kts": kts, "vbs": vbs,
                               "kst": kst, "vst": vst}

    return nc


def _in_maps(q, k, v):
    import ml_dtypes
    bf = ml_dtypes.bfloat16

    maps = []
    for c in range(NCORES):
        h0, h1 = c, c + 8
        s = (7 - c) % 8

        qT = np.concatenate([q[:, h0, :].T, q[:, h1, :].T], 0)  # [128, 4096]
        kT = np.concatenate([k[:, h0, :].T, k[:, h1, :].T], 0)
        qhm = np.zeros((1024, 896), np.float32)
        khm = np.zeros((1024, 512), np.float32)
        for t in range(8):
            w = min(896, SEQ - 512 * t)
            qhm[128 * t:128 * t + 128, :w] = qT[:, 512 * t:512 * t + w]
            khm[128 * t:128 * t + 128, :] = kT[:, 512 * t:512 * t + 512]

        vbm = np.zeros((1024, 1040), np.float32)
        for t in range(8):
            for a in range(4):
                blk = v[128 * (4 * t + a):128 * (4 * t + a) + 128, :, :]
                vbm[128 * t:128 * t + 128, 130 * a:130 * a + 64] = blk[:, h0, :]
                vbm[128 * t:128 * t + 128, 130 * a + 64] = 1.0
                vbm[128 * t:128 * t + 128, 130 * a + 65:130 * a + 129] = blk[:, h1, :]
                vbm[128 * t:128 * t + 128, 130 * a + 129] = 1.0
                # odd-block (tokens 64-127 of the pair) V dup at
                # partitions 0-63 for both heads (tail PV at col-group 0)
                col = 520 + 130 * a
                vbm[128 * t:128 * t + 64, col:col + 64] = blk[64:128, h0, :]
                vbm[128 * t:128 * t + 64, col + 64] = 1.0
                vbm[128 * t:128 * t + 64, col + 65:col + 129] = blk[64:128, h1, :]
                vbm[128 * t:128 * t + 64, col + 129] = 1.0

        ksm = np.zeros((128, 448), np.float32)
        for b in range(7):
            j = s + 8 * b
            ksm[0:64, 64 * b:64 * b + 64] = k[64 * j:64 * j + 64, h0, :].T
            ksm[64:128, 64 * b:64 * b + 64] = k[64 * j:64 * j + 64, h1, :].T

        vsm = np.zeros((128, 1040), np.float32)
        for p in range(4):  # pair 3 = lone block 6 on partitions 0-63
            halves = ((0, s + 16 * p), (1, s + 16 * p + 8)) if p < 3 \
                else ((0, s + 48),)
            for half, j in halves:
                r = slice(64 * half, 64 * half + 64)
                vsm[r, 130 * p:130 * p + 64] = v[64 * j:64 * j + 64, h0, :]
                vsm[r, 130 * p + 64] = 1.0
                vsm[r, 130 * p + 65:130 * p + 129] = v[64 * j:64 * j + 64, h1, :]
                vsm[r, 130 * p + 129] = 1.0
        for d in range(4):  # dup of even blocks (singles) on partitions 64-127
            j = s + 16 * d
            col = 130 * (4 + d)
            vsm[64:128, col:col + 64] = v[64 * j:64 * j + 64, h0, :]
            vsm[64:128, col + 64] = 1.0
            vsm[64:128, col + 65:col + 129] = v[64 * j:64 * j + 64, h1, :]
            vsm[64:128, col + 129] = 1.0

        biasm = np.zeros((1, 512), np.float32)
        biasm[0, :64 * s] = -1e9

        maps.append({"qh": qhm.astype(bf), "kh": khm.astype(bf),
                     "vbh": vbm.astype(bf), "ksh": ksm.astype(bf),
                     "vsh": vsm.astype(bf), "biasrow": biasm.astype(bf)})
    return maps


def kernel(q, k, v, cu_seqlens_k=None, **_):
    from concourse.bass_utils import run_bass_kernel_spmd

    q = np.asarray(q, np.float32)
    k = np.asarray(k, np.float32)
    v = np.asarray(v, np.float32)
    if "nc" not in _cache:
        _cache["nc"] = _legalize_waits(_build_program())
    res = run_bass_kernel_spmd(_cache["nc"], _in_maps(q, k, v),
                               list(range(NCORES))).results
    out = np.empty((SEQ, N_HEADS, HEAD), np.float32)
    for c in range(NCORES):
        o = np.asarray(res[c]["outh"], np.float32)
        sm = np.asarray(res[c]["sumh"], np.float32)
        out[:, c, :] = (o[0:64, :] / sm[0:1, :]).T
        out[:, c + 8, :] = (o[64:128, :] / sm[1:2, :]).T
    return out


# revision 33
# speedup vs baseline: 1.2326x; 1.1865x over previous
"""Block-sparse local+strided attention (LocalStridedBlockSparseAttn) on 8 trn2 cores.

q,k,v [4096, 16, 64] f32, single prefill sequence. Per-head block mask (64x64
token blocks): j <= i and (i - j < 8 or (j + h + 1) % 8 == 0). Core c owns
heads {c, c+8}; both share the strided residue s = (7 - c) % 8, so one SPMD
program serves all 8 cores with per-core data only.

v2 design (bf16, PE-lean):
  - Host pre-transposes q,k to [128 = 2 heads x 64 d, T] bf16 and pre-packs V
    with ones columns; normalization (divide by row sums) happens on host.
  - QK produces P^T [k-tokens, q] tiles directly: stationary = kT slices
    (d=64 on partitions -> the two heads run CONCURRENTLY on disjoint
    PE row groups via tile_position auto-derivation), moving = qT (N=512).
  - Local band processed as 32 k-pair windows: pair m (128 k-tokens) vs
    q cols [128m, 128m+512) in one N<=512 matmul + a [64,64] tail
    (j=2m+1 vs i=2m+8). Diagonal causal masking post-exp via one
    strided-AP multiply per chunk-head on gpsimd.
  - Strided part per 512-q chunk c: packed k-block pairs (N=512), the
    boundary block's partial validity handled by a -1e9 bias row input
    (per-core data) accumulated via a [1,*] matmul pre-exp.
  - exp split: ACT (exact, table exp) for strided/singles/edge mains;
    DVE computes exp via a Schraudolph bf16-bitcast (one tensor_scalar:
    int16(A*s + B) reinterpreted as bf16) for most local mains + tails.
  - PV in O^T layout: stationary Vaug = [V | 1] (65 cols), moving = P^T
    pieces, accumulated per chunk into PSUM [65, 512]; row 64 = softmax
    denominators. Copy to SBUF, DMA out; host divides and transposes.
"""

import numpy as np

N_HEADS = 16
HEAD = 64
SEQ = 4096
BS = 64
NB = 64
CHUNK = 512
NCH = SEQ // CHUNK   # 8
NCORES = 8
NPAIR = 32           # local k-pairs (128 tokens each)
SM_SCALE = 0.125

# Schraudolph exp in bf16 bit space: bf16bits(exp(s/8)) ~ int(A*s + B)
SCH_A = 128.0 / float(np.log(2.0)) * SM_SCALE      # 23.0831...
SCH_B = 16256.0 - 7.5 + 0.5                        # bias - RMS-centering + trunc comp

_cache = {}

import os
EXP_SPLIT_BANKS = os.environ.get("KBS_EXP_SPLIT", "0") == "1"   # per-bank exp APs
USE_SCHRAUDOLPH = os.environ.get("KBS_SCH", "1") == "1"         # DVE int16 exp
MASK_ON_GPSIMD = os.environ.get("KBS_GPSIMD", "1") == "1"       # gpsimd mask mult
NCH_RUN = int(os.environ.get("KBS_NCH", "8"))                   # chunks to emit
NO_PV = os.environ.get("KBS_NO_PV", "0") == "1"
NO_MASK = os.environ.get("KBS_NO_MASK", "0") == "1"
NO_TAILS = os.environ.get("KBS_NO_TAILS", "0") == "1"
NO_STR = os.environ.get("KBS_NO_STR", "0") == "1"
TAILS_H = os.environ.get("KBS_TAILS_H", "01")    # which heads get tails


def _legalize_waits(nc, max_waits=1):
    """This walrus build rejects instructions carrying more than one sync-wait
    condition; hoist extras into same-engine NoOps placed before the instr."""
    import concourse.mybir as mybir

    nid = 0
    for bb in nc.main_func.blocks:
        new = []
        for ins in bb.instructions:
            si = ins.sync_info
            if si is not None and si.on_wait and len(si.on_wait) > max_waits:
                waits = list(si.on_wait)
                while len(waits) > max_waits:
                    chunk, waits = waits[:max_waits], waits[max_waits:]
                    nid += 1
                    nop = mybir.InstNoOp(name=f"{ins.name}-wsplit{nid}")
                    nop.engine = ins.engine
                    nop.sync_info = mybir.SyncInfo(on_wait=chunk, on_update=[])
                    new.append(nop)
                ins.sync_info = mybir.SyncInfo(on_wait=waits,
                                               on_update=list(si.on_update))
            new.append(ins)
        bb.instructions[:] = new
    return nc


def _mask_np():
    """[128, 512] = 4 copies of the 128x128 diag-pair mask: rows 0-63 (even
    block j=2m) vs q blocks (2m, 2m+1): [tril-allow | ones]; rows 64-127
    (j=2m+1): [zeros | tril-allow]."""
    n = np.arange(64)
    tri = (n[None, :] >= n[:, None]).astype(np.float32)  # q >= k within block
    m = np.zeros((128, 128), np.float32)
    m[0:64, 0:64] = tri
    m[0:64, 64:128] = 1.0
    m[64:128, 64:128] = tri
    return np.tile(m, (1, 4))


def _build_program():
    from contextlib import ExitStack

    import concourse.bass as bass
    import concourse.mybir as mybir
    from concourse import tile

    f32 = mybir.dt.float32
    bf16 = mybir.dt.bfloat16
    i16 = mybir.dt.int16
    Exp = mybir.ActivationFunctionType.Exp
    MUL = mybir.AluOpType.mult
    ADD = mybir.AluOpType.add

    nc = bass.Bass()
    qh = nc.dram_tensor("qh", [8 * 128, 896], bf16, kind="ExternalInput")
    kh = nc.dram_tensor("kh", [8 * 128, 512], bf16, kind="ExternalInput")
    vbh = nc.dram_tensor("vbh", [8 * 128, 1040], bf16, kind="ExternalInput")
    ksh = nc.dram_tensor("ksh", [128, 448], bf16, kind="ExternalInput")
    vsh = nc.dram_tensor("vsh", [128, 1040], bf16, kind="ExternalInput")
    biasrow = nc.dram_tensor("biasrow", [1, 512], bf16, kind="ExternalInput")
    outh = nc.dram_tensor("outh", [128, SEQ], f32, kind="ExternalOutput")
    sumh = nc.dram_tensor("sumh", [2, SEQ], f32, kind="ExternalOutput")

    import ml_dtypes
    mask_d = nc.inline_tensor(_mask_np().astype(ml_dtypes.bfloat16), "mask_c")
    ones_d = nc.inline_tensor(np.ones((1, 128), ml_dtypes.bfloat16), "ones_c")

    with tile.TileContext(nc) as tc, ExitStack() as ctx:
        const = ctx.enter_context(tc.tile_pool(name="const", bufs=1))
        mask = const.tile([128, 512], bf16, tag="mask")
        ones = const.tile([1, 128], bf16, tag="ones")
        bias = const.tile([1, 512], bf16, tag="bias")
        kst = const.tile([128, 448], bf16, tag="kst")
        vst = const.tile([128, 1040], bf16, tag="vst")
        qpool = ctx.enter_context(tc.tile_pool(name="qts", bufs=8))
        kpool = ctx.enter_context(tc.tile_pool(name="kts", bufs=8))
        vpool = ctx.enter_context(tc.tile_pool(name="vbs", bufs=8))
        qts, kts, vbs = [], [], []
        for t in range(NCH):
            qts.append(qpool.tile([128, 896], bf16, tag="qts",
                                  name=f"qt{t}"))
            kts.append(kpool.tile([128, 512], bf16, tag="kts",
                                  name=f"kt{t}"))
            vbs.append(vpool.tile([128, 1040], bf16, tag="vbs",
                                  name=f"vt{t}"))
        # chunk-0 q/k first on the two HWDGE queues (SP + ACT) so the PE
        # can start ASAP; consts mid-stream
        nc.sync.dma_start(qts[0][:], qh[0:128, :])
        nc.scalar.dma_start(kts[0][:], kh[0:128, :])
        nc.scalar.dma_start(mask[:], mask_d[:])
        nc.sync.dma_start(vbs[0][:], vbh[0:128, :])
        nc.scalar.dma_start(qts[1][:], qh[128:256, :])
        nc.sync.dma_start(kts[1][:], kh[128:256, :])
        nc.scalar.dma_start(ones[:], ones_d[:])
        nc.scalar.dma_start(bias[:], biasrow[:])
        nc.sync.dma_start(kst[:], ksh[:])
        nc.sync.dma_start(vst[:], vsh[:])
        for t in range(2, NCH):
            nc.sync.dma_start(qts[t][:], qh[128 * t:128 * (t + 1), :])
            nc.sync.dma_start(kts[t][:], kh[128 * t:128 * (t + 1), :])
            nc.sync.dma_start(vbs[t - 1][:], vbh[128 * (t - 1):128 * t, :])
        nc.sync.dma_start(vbs[7][:], vbh[128 * 7:128 * 8, :])

        # SBUF P-hat pools
        pmpool = ctx.enter_context(tc.tile_pool(name="pmain", bufs=3))
        pspool_sb = ctx.enter_context(tc.tile_pool(name="pstr", bufs=2))
        pgpool_sb = ctx.enter_context(tc.tile_pool(name="psng", bufs=2))
        ptpool_sb = ctx.enter_context(tc.tile_pool(name="ptail", bufs=2))
        osbpool = ctx.enter_context(tc.tile_pool(name="osb", bufs=4))
        # PSUM pools
        psp = ctx.enter_context(tc.tile_pool(name="ps", bufs=2, space="PSUM"))
        otp = ctx.enter_context(tc.tile_pool(name="ot", bufs=3, space="PSUM"))
        ptp = ctx.enter_context(tc.tile_pool(name="pt", bufs=1, space="PSUM"))
        # banks: ps 2x2 + ot 4x1 = 8 (tails share the ps slots)

        pmains = [None] * NCH   # [128, 4096] bf16: pair 4c+j at [1024j:+1024]
        ptails = [None] * NCH   # [128, 512] bf16 (rows 64-127 used)
        pstrs = [None] * NCH    # [128, 3072] bf16: stack p at [1024p:+1024]
        psngs = [None] * NCH    # [128, 512] bf16

        def emit_qk_mains(c, jlist):
            pm = pmains[c]
            for j in jlist:
                m = 4 * c + j
                W = min(512, SEQ - 128 * m)
                ps = psp.tile([128, 1024], f32, tag="ps", name=f"ps_m{m}")
                for h in (0, 1):
                    hp = slice(64 * h, 64 * h + 64)
                    nc.tensor.matmul(
                        ps[:, 512 * h:512 * h + W],
                        kts[c][hp, 128 * j:128 * j + 128],
                        qts[c][hp, 128 * j:128 * j + W],
                        start=True, stop=True, skip_group_check=True)
                if j < 3 and m <= 28 and USE_SCHRAUDOLPH:
                    if EXP_SPLIT_BANKS:
                        for h in (0, 1):
                            nc.vector.tensor_scalar(
                                pm[:, 1024 * j + 512 * h:1024 * j + 512 * h + 512].bitcast(i16),
                                ps[:, 512 * h:512 * h + 512],
                                SCH_A, SCH_B, MUL, ADD)
                    else:
                        dst = pm[:, 1024 * j:1024 * j + 1024]
                        nc.vector.tensor_scalar(dst.bitcast(i16), ps[:],
                                                SCH_A, SCH_B, MUL, ADD)
                elif W == 512:
                    if EXP_SPLIT_BANKS:
                        for h in (0, 1):
                            nc.scalar.activation(
                                pm[:, 1024 * j + 512 * h:1024 * j + 512 * h + 512],
                                ps[:, 512 * h:512 * h + 512], Exp, scale=SM_SCALE)
                    else:
                        dst = pm[:, 1024 * j:1024 * j + 1024]
                        nc.scalar.activation(dst, ps[:], Exp, scale=SM_SCALE)
                else:
                    # edge pairs: only [0:W] / [512:512+W] were written
                    for h in (0, 1):
                        nc.scalar.activation(
                            pm[:, 1024 * j + 512 * h:1024 * j + 512 * h + W],
                            ps[:, 512 * h:512 * h + W], Exp, scale=SM_SCALE)
                if not NO_MASK:
                    # zero the diag-masked regions (cols [0:128] of each
                    # head's half) in one 2D-strided multiply on DVE
                    src2 = pm[:, 1024 * j:1024 * j + 1024].rearrange(
                        "p (g w) -> p g w", w=512)[:, :, 0:128]
                    msk2 = mask[:, 0:256].rearrange("p (g w) -> p g w", w=128)
                    nc.gpsimd.tensor_tensor(src2, src2, msk2, MUL)

        def emit_qk_rest(c):
            # strided pairs (blocks 2p, 2p+1 valid: 2p+1 <= c-1)
            npairs = c // 2
            for p in range(npairs):
                bnd = (c % 2 == 0) and (p == npairs - 1)   # block c-1 == 2p+1
                ps = psp.tile([128, 1024], f32, tag="ps", name=f"ps_s{c}_{p}")
                for h in (0, 1):
                    hp = slice(64 * h, 64 * h + 64)
                    nc.tensor.matmul(
                        ps[:, 512 * h:512 * h + 512],
                        kst[hp, 128 * p:128 * p + 128],
                        qts[c][hp, 0:512],
                        start=True, stop=not bnd, skip_group_check=True)
                    if bnd:
                        nc.tensor.matmul(
                            ps[64:128, 512 * h:512 * h + 512],
                            ones[0:1, 0:64], bias[:],
                            start=False, stop=True, skip_group_check=True)
                if EXP_SPLIT_BANKS:
                    for h in (0, 1):
                        nc.scalar.activation(
                            pstrs[c][:, 1024 * p + 512 * h:1024 * p + 512 * h + 512],
                            ps[:, 512 * h:512 * h + 512], Exp, scale=SM_SCALE)
                else:
                    nc.scalar.activation(pstrs[c][:, 1024 * p:1024 * p + 1024],
                                         ps[:], Exp, scale=SM_SCALE)
            if c % 2 == 1:
                # single strided block b = c-1: h0 in bank A rows 0-63,
                # h1 in bank B rows 64-127 (independent start=True per bank)
                b = c - 1
                pg = psp.tile([128, 1024], f32, tag="ps", name=f"ps_g{c}")
                for h in (0, 1):
                    hp = slice(64 * h, 64 * h + 64)
                    reg = pg[hp, 512 * h:512 * h + 512]
                    nc.tensor.matmul(reg, kst[hp, 64 * b:64 * b + 64],
                                     qts[c][hp, 0:512],
                                     start=True, stop=False,
                                     skip_group_check=True)
                    nc.tensor.matmul(reg, ones[0:1, 0:64], bias[:],
                                     start=False, stop=True,
                                     skip_group_check=True)
                    nc.scalar.activation(psngs[c][hp, :], reg, Exp,
                                         scale=SM_SCALE)

        def emit_qk_tails(c):
            # tails m = 4(c-1)+j: j(block 2m+1) vs i = 2m+8 (q-block in chunk c)
            # h0 tails rows 0-63 (tile (0,0)); h1 rows 64-127 (tile
            # (64,64)). start=True clears only the tile's own PSUM partition
            # range, so per-head start=True shares one bank safely.
            pt = ptp.tile([128, 512], f32, tag="pt", name=f"pt{c}")
            from concourse.tile_rust import add_dep_helper
            hlist = [int(x) for x in TAILS_H]
            mm0h = {}
            for j in range(4):
                for h in hlist:
                    hp = slice(64 * h, 64 * h + 64)
                    col = 256 * h + 64 * j
                    mm = nc.tensor.matmul(
                        pt[hp, col:col + 64],
                        kts[c - 1][hp, 128 * j + 64:128 * j + 128],
                        qts[c][hp, 128 * j:128 * j + 64],
                        start=(j == 0), stop=(j == 3),
                        skip_group_check=True)
                    if j == 0:
                        mm0h[h] = mm
                    else:
                        # j=0's start=True clears the tile's psum range; the
                        # disjoint-region matmuls must not be scheduled first
                        add_dep_helper(mm.ins, mm0h[h].ins, sync=False,
                                       reason="tail has_written order")
            for h in hlist:
                hp = slice(64 * h, 64 * h + 64)
                dst = ptails[c][hp, 256 * h:256 * h + 256]
                src_ = pt[hp, 256 * h:256 * h + 256]
                if USE_SCHRAUDOLPH:
                    nc.vector.tensor_scalar(dst.bitcast(i16), src_,
                                            SCH_A, SCH_B, MUL, ADD)
                else:
                    nc.scalar.activation(dst, src_, Exp, scale=SM_SCALE)

        def emit_mask(c):
            # zero the masked diag regions of the 4 mains, one op per head
            pm = pmains[c]
            src = pm[:].rearrange("p (j w) -> p j w", w=1024)
            msk = mask[:].rearrange("p (j w) -> p j w", w=128)
            eng = nc.gpsimd if MASK_ON_GPSIMD else nc.vector
            for h in (0, 1):
                ap = src[:, :, 512 * h:512 * h + 128]
                eng.tensor_tensor(ap, ap, msk[:, :, :], MUL)

        def emit_pv(c, heads=(0, 1)):
            for h in heads:
                pieces = []   # (lhsT, rhs, out_col, W) - 512-wide ones first
                pm = pmains[c]
                # pair 4c covers the full chunk (chain-A opener)
                pieces.append((vbs[c][:, 65 * h:65 * h + 65],
                               pm[:, 512 * h:512 * h + 512], 0, 512))
                for p in range(c // 2 if not NO_STR else 0):
                    pieces.append((vst[:, 130 * p + 65 * h:130 * p + 65 * h + 65],
                                   pstrs[c][:, 1024 * p + 512 * h:1024 * p + 512 * h + 512],
                                   0, 512))
                if c % 2 == 1 and not NO_STR:
                    b = c - 1
                    if h == 0:
                        pieces.append((vst[0:64, 130 * (b // 2):130 * (b // 2) + 65],
                                       psngs[c][0:64, :], 0, 512))
                    else:
                        pieces.append((vst[64:128, 130 * (4 + b // 2) + 65:130 * (4 + b // 2) + 130],
                                       psngs[c][64:128, :], 0, 512))
                for j in range(1, 4):
                    m = 4 * c + j
                    if m > 31:
                        continue
                    W = 512 - 128 * j
                    pieces.append((vbs[c][:, 130 * j + 65 * h:130 * j + 65 * h + 65],
                                   pm[:, 1024 * j + 512 * h:1024 * j + 512 * h + W],
                                   128 * j, W))
                if c >= 1:
                    pmp = pmains[c - 1]
                    for j in range(1, 4):
                        W = 128 * j
                        off = 1024 * j + 512 * h + (512 - W)
                        pieces.append((vbs[c - 1][:, 130 * j + 65 * h:130 * j + 65 * h + 65],
                                       pmp[:, off:off + W], 0, W))
                    tails_on = (not NO_TAILS) and str(h) in TAILS_H
                    for j in range(4 if tails_on else 0):  # tails
                        if h == 0:
                            pieces.append((vbs[c - 1][0:64, 520 + 130 * j:520 + 130 * j + 65],
                                           ptails[c][0:64, 64 * j:64 * j + 64],
                                           128 * j, 64))
                        else:
                            pieces.append((vbs[c - 1][64:128, 130 * j + 65:130 * j + 130],
                                           ptails[c][64:128, 256 + 64 * j:256 + 64 * j + 64],
                                           128 * j, 64))
                # alternate pieces across two accumulator banks so consecutive
                # accumulating matmuls never chain through the same PSUM bank
                two = False  # bank-alternation regressed; single chain
                ota = otp.tile([65, 512], f32, tag="ot", name=f"ot{c}_{h}a")
                otb = otp.tile([65, 512], f32, tag="ot", name=f"ot{c}_{h}b") \
                    if two else None
                chains = ([], [])
                for pi, pc_ in enumerate(pieces):
                    chains[pi % 2 if two else 0].append(pc_)
                # emit interleaved A0,B0,A1,B1,... so consecutive PE matmuls
                # target different banks and their drains overlap
                for pi, (vl, rh, col, W) in enumerate(pieces):
                    ci = pi % 2 if two else 0
                    tgt = (ota, otb)[ci]
                    chain = chains[ci]
                    idx = pi // 2 if two else pi
                    nc.tensor.matmul(tgt[:, col:col + W], vl, rh,
                                     start=(idx == 0),
                                     stop=(idx == len(chain) - 1),
                                     skip_group_check=True)
                osb = osbpool.tile([65, 512], f32, tag="osb",
                                   name=f"osb{c}_{h}")
                if two:
                    # only one PSUM operand allowed per instruction
                    nc.scalar.copy(osb[:], ota[:])
                    nc.vector.tensor_tensor(osb[:], osb[:], otb[:], ADD)
                else:
                    nc.any.tensor_copy(osb[:], ota[:])
                nc.sync.dma_start(outh[64 * h:64 * h + 64,
                                       512 * c:512 * c + 512], osb[0:64, :])
                nc.sync.dma_start(sumh[h:h + 1, 512 * c:512 * c + 512],
                                  osb[64:65, :])

        for c in range(NCH_RUN):
            pmains[c] = pmpool.tile([128, 4096], bf16, tag="pmain",
                                    name=f"pm{c}")
            if c >= 1:
                pstrs[c] = pspool_sb.tile([128, 3072], bf16, tag="pstr",
                                          name=f"pstr{c}")
                ptails[c] = ptpool_sb.tile([128, 512], bf16, tag="ptail",
                                           name=f"ptl{c}")
            if c % 2 == 1:
                psngs[c] = pgpool_sb.tile([128, 512], bf16, tag="psng",
                                          name=f"psg{c}")
            emit_qk_mains(c, (0, 1))
            if c >= 1 and not NO_PV:
                emit_pv(c - 1, (0,))
            emit_qk_mains(c, (2, 3))
            if c >= 1 and not NO_PV:
                emit_pv(c - 1, (1,))
            if c >= 1 and not NO_TAILS:
                emit_qk_tails(c)
            if not NO_STR:
                emit_qk_rest(c)

        if not NO_PV:
            emit_pv(NCH_RUN - 1)
        _cache["dbg_tiles"] = {"pmains": pmains, "ptails": ptails,
                               "pstrs": pstrs, "psngs": psngs,
                               "qts": qts, "kts": kts, "vbs": vbs,
                               "kst": kst, "vst": vst}

    return nc


def _in_maps(q, k, v):
    import ml_dtypes
    bf = ml_dtypes.bfloat16

    maps = []
    for c in range(NCORES):
        h0, h1 = c, c + 8
        s = (7 - c) % 8

        qT = np.concatenate([q[:, h0, :].T, q[:, h1, :].T], 0)  # [128, 4096]
        kT = np.concatenate([k[:, h0, :].T, k[:, h1, :].T], 0)
        qhm = np.zeros((1024, 896), np.float32)
        khm = np.zeros((1024, 512), np.float32)
        for t in range(8):
            w = min(896, SEQ - 512 * t)
            qhm[128 * t:128 * t + 128, :w] = qT[:, 512 * t:512 * t + w]
            khm[128 * t:128 * t + 128, :] = kT[:, 512 * t:512 * t + 512]

        vbm = np.zeros((1024, 1040), np.float32)
        for t in range(8):
            for a in range(4):
                blk = v[128 * (4 * t + a):128 * (4 * t + a) + 128, :, :]
                vbm[128 * t:128 * t + 128, 130 * a:130 * a + 64] = blk[:, h0, :]
                vbm[128 * t:128 * t + 128, 130 * a + 64] = 1.0
                vbm[128 * t:128 * t + 128, 130 * a + 65:130 * a + 129] = blk[:, h1, :]
                vbm[128 * t:128 * t + 128, 130 * a + 129] = 1.0
                # odd-block (tokens 64-127 of the pair) V dup at
                # partitions 0-63 for both heads (tail PV at col-group 0)
                col = 520 + 130 * a
                vbm[128 * t:128 * t + 64, col:col + 64] = blk[64:128, h0, :]
                vbm[128 * t:128 * t + 64, col + 64] = 1.0
                vbm[128 * t:128 * t + 64, col + 65:col + 129] = blk[64:128, h1, :]
                vbm[128 * t:128 * t + 64, col + 129] = 1.0

        ksm = np.zeros((128, 448), np.float32)
        for b in range(7):
            j = s + 8 * b
            ksm[0:64, 64 * b:64 * b + 64] = k[64 * j:64 * j + 64, h0, :].T
            ksm[64:128, 64 * b:64 * b + 64] = k[64 * j:64 * j + 64, h1, :].T

        vsm = np.zeros((128, 1040), np.float32)
        for p in range(4):  # pair 3 = lone block 6 on partitions 0-63
            halves = ((0, s + 16 * p), (1, s + 16 * p + 8)) if p < 3 \
                else ((0, s + 48),)
            for half, j in halves:
                r = slice(64 * half, 64 * half + 64)
                vsm[r, 130 * p:130 * p + 64] = v[64 * j:64 * j + 64, h0, :]
                vsm[r, 130 * p + 64] = 1.0
                vsm[r, 130 * p + 65:130 * p + 129] = v[64 * j:64 * j + 64, h1, :]
                vsm[r, 130 * p + 129] = 1.0
        for d in range(4):  # dup of even blocks (singles) on partitions 64-127
            j = s + 16 * d
            col = 130 * (4 + d)
            vsm[64:128, col:col + 64] = v[64 * j:64 * j + 64, h0, :]
            vsm[64:128, col + 64] = 1.0
            vsm[64:128, col + 65:col + 129] = v[64 * j:64 * j + 64, h1, :]
            vsm[64:128, col + 129] = 1.0

        biasm = np.zeros((1, 512), np.float32)
        biasm[0, :64 * s] = -1e9

        maps.append({"qh": qhm.astype(bf), "kh": khm.astype(bf),
                     "vbh": vbm.astype(bf), "ksh": ksm.astype(bf),
                     "vsh": vsm.astype(bf), "biasrow": biasm.astype(bf)})
    return maps


def kernel(q, k, v, cu_seqlens_k=None, **_):
    from concourse.bass_utils import run_bass_kernel_spmd

    q = np.asarray(q, np.float32)
    k = np.asarray(k, np.float32)
    v = np.asarray(v, np.float32)
    if "nc" not in _cache:
        _cache["nc"] = _legalize_waits(_build_program())
    res = run_bass_kernel_spmd(_cache["nc"], _in_maps(q, k, v),
                               list(range(NCORES))).results
    out = np.empty((SEQ, N_HEADS, HEAD), np.float32)
    for c in range(NCORES):
        o = np.asarray(res[c]["outh"], np.float32)
        sm = np.asarray(res[c]["sumh"], np.float32)
        out[:, c, :] = (o[0:64, :] / sm[0:1, :]).T
        out[:, c + 8, :] = (o[64:128, :] / sm[1:2, :]).T
    return out
